# revision 13
# baseline (speedup 1.0000x reference)
"""Trainium2 Bass kernel for nn_CrossModalFusion (single-head cross attention).

Per-batch-element cross attention, data-parallel over B=8 across 8 NeuronCores.

Math (per core, T=2048, D_RGB=400, D_POSE=256, H=512):
    S = q k^T = (x Wq + bq)(p Wk + bk)^T
      = x (Wq Wk^T) p^T + s1[tq] + s2[tk] + c0
    with M = Wq Wk^T (rank<=256!), s1 = x (Wq bk), s2 = p (Wk bq),
    c0 = bq.bk. The s1/c0 terms are constant along the softmax axis and
    cancel exactly in y_un/rowsum; only s2 (per-key) must be kept.
    So scores contract over 256 (pose dim) instead of 512:
        u = x M            (device, fp8 DoubleRow, contract 400->512pad)
        ST = p u^T         (one 256-contract DR matmul per [128,512] tile)
        A' = exp(SCALE*ST + SCALE*s2 - ln 32)  (ACT; bias host-computed)
    v = p Wv ; O^T = v^T A' ; rowsum = 1^T A' ; y_un^T = O^T-> y = rgb +
    bp + bv Wp + (A'@v)@Wp / rowsum(A') applied on host (exact fp32).

Perf model (measured): PE matmul issue rate = out_free_dim * 0.43ns
regardless of perf mode / LDWEIGHTS (weight loads hide in the reorder
window). So minimize sum(free) over matmuls: low-rank scores halve the
score matmuls and kill the q/k projections and their PSUM evictions.
Output shipped bf16 (halves the output DMA); host divides in fp32.
"""

import sys

if "/opt/trn_rl_repo" not in sys.path:
    sys.path.insert(0, "/opt/trn_rl_repo")

from contextlib import ExitStack

import ml_dtypes
import numpy as np

import concourse.mybir as mybir
import concourse.tile as tile
from concourse import bacc, bass_utils

FP8 = mybir.dt.float8e4
F32 = mybir.dt.float32
BF16 = mybir.dt.bfloat16
NP_FP8 = ml_dtypes.float8_e4m3
NP_BF16 = ml_dtypes.bfloat16

B, T, DR, DP, H = 8, 2048, 400, 256, 512
PART = 128
DRP = 512                # rgb feature dim padded to 4*128
TQC = 512                # tq chunk width (max PSUM free dim)
NCH = T // TQC           # 4 chunks
NTK = T // PART          # 16 key tiles
NKP = NTK // 2           # 8 key tile pairs (DoubleRow)
NHT = H // PART          # 4 h tiles
NHP = NHT // 2           # 2 h tile pairs
NDR = DRP // PART        # 4 padded-rgb d tiles
NDRP = NDR // 2          # 2 pairs
NDP = DP // PART         # 2 pose d tiles
SCALE = float(1.0 / np.sqrt(np.float32(H)))
EXP_BIAS = float(-np.log(32.0))

AT = mybir.ActivationFunctionType
OP = mybir.AluOpType
DRM = mybir.MatmulPerfMode.DoubleRow


def build_nc():
    nc = bacc.Bacc(
        "TRN2",
        target_bir_lowering=False,
        debug=False,
        enable_asserts=False,
        num_devices=8,
    )
    xT = nc.dram_tensor("xT", (DRP, T), FP8, kind="ExternalInput").ap()
    pT = nc.dram_tensor("pT", (DP, T), FP8, kind="ExternalInput").ap()
    m8d = nc.dram_tensor("m8d", (DRP, DP), FP8, kind="ExternalInput").ap()
    wv = nc.dram_tensor("wv", (DP, H), FP8, kind="ExternalInput").ap()
    wp = nc.dram_tensor("wp", (H, DR), FP8, kind="ExternalInput").ap()
    biasd = nc.dram_tensor("biasd", (PART, NTK), F32, kind="ExternalInput").ap()
    yun = nc.dram_tensor("yun", (T, DR), BF16, kind="ExternalOutput").ap()
    sums_out = nc.dram_tensor("sums_out", (NCH, TQC), F32, kind="ExternalOutput").ap()

    with tile.TileContext(nc) as tc, ExitStack() as ctx:
        const = ctx.enter_context(tc.tile_pool(name="const", bufs=1))
        mm_ps = ctx.enter_context(tc.tile_pool(name="mm_ps", bufs=3, space="PSUM"))
        ot_ps = ctx.enter_context(tc.tile_pool(name="ot_ps", bufs=4, space="PSUM"))
        sum_ps = ctx.enter_context(tc.tile_pool(name="sum_ps", bufs=1, space="PSUM"))
        ex_pool = ctx.enter_context(tc.tile_pool(name="ex_pool", bufs=10))
        sums_pool = ctx.enter_context(tc.tile_pool(name="sums_pool", bufs=2))
        ysb_pool = ctx.enter_context(tc.tile_pool(name="ysb_pool", bufs=4))

        # ---- persistent inputs ----
        # x-half-0 + m8 first (they gate the uT matmuls which start phase A);
        # pose quarters spread across sync/gpsimd so v tiles stream in behind
        m8 = const.tile([PART, NDR, DP], FP8, name="m8")
        nc.scalar.dma_start(m8[:], m8d.rearrange("(k p) e -> p k e", p=PART))
        x8 = const.tile([PART, NDR, T], FP8, name="x8")
        nc.sync.dma_start(
            x8[:, :, 0 : T // 2],
            xT[:, 0 : T // 2].rearrange("(k p) t -> p k t", p=PART),
        )
        wv8 = const.tile([PART, NDP, H], FP8, name="wv8")
        nc.scalar.dma_start(wv8[:], wv.rearrange("(k p) h -> p k h", p=PART))
        bias_sb = const.tile([PART, NTK], F32, name="bias_sb")
        nc.gpsimd.dma_start(bias_sb[:], biasd[:])
        p8 = const.tile([PART, NDP, T], FP8, name="p8")
        for h, eng in ((0, nc.sync), (1, nc.gpsimd), (2, nc.sync), (3, nc.gpsimd)):
            eng.dma_start(
                p8[:, :, h * (T // 4) : (h + 1) * (T // 4)],
                pT[:, h * (T // 4) : (h + 1) * (T // 4)].rearrange(
                    "(k p) t -> p k t", p=PART
                ),
            )
        nc.scalar.dma_start(
            x8[:, :, T // 2 : T],
            xT[:, T // 2 : T].rearrange("(k p) t -> p k t", p=PART),
        )
        wp8 = const.tile([PART, NHT, DR], FP8, name="wp8")
        nc.scalar.dma_start(wp8[:], wp.rearrange("(k p) d -> p k d", p=PART))
        ones8 = const.tile([PART, 2, 16], FP8, name="ones8")
        nc.vector.memset(ones8[:], 1.0)

        # ---- persistent intermediates (fp8 DoubleRow pair layouts) ----
        # uT8[p, s, t] = u[e = s*128 + p, t] where u = x @ M (e in 0..255)
        uT8 = const.tile([PART, NDP, T], FP8, name="uT8")
        # v8[j2][p, s, h] = v[t = j2*256 + s*128 + p, h]
        v8 = [const.tile([PART, 2, H], FP8, name=f"v8_{j}") for j in range(NKP)]
        # ot8[i2][p, s, t] = O[h = i2*256 + s*128 + p, t] (unnormalized, /32)
        ot8 = [const.tile([PART, 2, T], FP8, name=f"ot8_{i}") for i in range(NHP)]

        # ---- phase A: u = x @ M and v projection (both fp8 DoubleRow) ----
        # ACT is exp-free until the chunk loop starts, so ph-A evictions
        # alternate ACT/DVE. Order: uT chunks 0-1 (gated on x half 0), then
        # v (gated on pose quarters), then uT chunks 2-3 (x half 1).
        def emit_ut(c):
            for e in range(NDP):
                ps = mm_ps.tile([PART, TQC], F32, name=f"ups_{e}_{c}", tag="mmps")
                for d2 in range(NDRP):
                    nc.tensor.matmul(
                        ps[:],
                        m8[:, 2 * d2 : 2 * d2 + 2, e * PART : (e + 1) * PART],
                        x8[:, 2 * d2 : 2 * d2 + 2, c * TQC : (c + 1) * TQC],
                        start=(d2 == 0),
                        stop=(d2 == NDRP - 1),
                        perf_mode=DRM,
                    )
                dst = uT8[:, e, c * TQC : (c + 1) * TQC]
                if (c * NDP + e) % 2 == 0:
                    nc.vector.tensor_copy(dst, ps[:])
                else:
                    nc.scalar.copy(dst, ps[:])

        for c in range(2):
            emit_ut(c)
        # v[t,h] = pT[d,t].T @ Wv[d,h] -> fp8
        for j in range(NTK):
            ps = mm_ps.tile([PART, H], F32, name=f"vps_{j}", tag="mmps")
            nc.tensor.matmul(
                ps[:],
                p8[:, :, j * PART : (j + 1) * PART],
                wv8[:],
                start=True,
                stop=True,
                perf_mode=DRM,
            )
            if j % 2 == 0:
                nc.scalar.copy(v8[j // 2][:, j % 2, :], ps[:])
            else:
                nc.vector.tensor_copy(v8[j // 2][:, j % 2, :], ps[:])
        for c in range(2, NCH):
            emit_ut(c)

        # ---- phase B: attention, chunked over tq ----
        # phase C (output projection) for chunk c-1 is emitted a few j-steps
        # into chunk c so its PSUM/engine traffic doesn't cluster at the
        # chunk boundary.
        def emit_y_tile(c, tl):
            tg = c * (TQC // PART) + tl
            yp = mm_ps.tile([PART, DR], F32, name=f"yp_{tg}", tag="mmps")
            for i2 in range(NHP):
                nc.tensor.matmul(
                    yp[:],
                    ot8[i2][:, :, tg * PART : (tg + 1) * PART],
                    wp8[:, 2 * i2 : 2 * i2 + 2, :],
                    start=(i2 == 0),
                    stop=(i2 == NHP - 1),
                    perf_mode=DRM,
                )
            ysb = ysb_pool.tile([PART, DR], BF16, name=f"ysb_{tg}", tag="ysb")
            # alternate engines so the final chain parallelizes at kernel end
            if tl % 2 == 0:
                nc.vector.tensor_copy(ysb[:], yp[:])
            else:
                nc.scalar.copy(ysb[:], yp[:])
            eng = nc.sync if tl % 2 == 0 else nc.gpsimd
            eng.dma_start(yun[tg * PART : (tg + 1) * PART, :], ysb[:])

        for c in range(NCH):
            otps = [
                ot_ps.tile([PART, TQC], F32, name=f"otp_{c}_{i}", tag="otp")
                for i in range(NHT)
            ]
            sps = sum_ps.tile([1, TQC], F32, name=f"sump_{c}", tag="sump")
            exs = []

            def emit_group(j2):
                # sums + A@v accumulation for key-tile pair j2; emitted one
                # j-pair behind the exp that produces ex[j2], so the ACT->PE
                # handoff is never on the PE critical path
                ex = exs[j2]
                nc.tensor.matmul(
                    sps[:],
                    ones8[:, :, 0:1],
                    ex[:],
                    start=(j2 == 0),
                    stop=(j2 == NKP - 1),
                    perf_mode=DRM,
                )
                for i in range(NHT):
                    nc.tensor.matmul(
                        otps[i][:],
                        v8[j2][:, :, i * PART : (i + 1) * PART],
                        ex[:],
                        start=(j2 == 0),
                        stop=(j2 == NKP - 1),
                        perf_mode=DRM,
                    )

            for j in range(NTK):
                st = mm_ps.tile([PART, TQC], F32, name=f"st_{c}_{j}", tag="mmps")
                nc.tensor.matmul(
                    st[:],
                    p8[:, :, j * PART : (j + 1) * PART],
                    uT8[:, :, c * TQC : (c + 1) * TQC],
                    start=True,
                    stop=True,
                    perf_mode=DRM,
                )
                if j % 2 == 0:
                    ex = ex_pool.tile([PART, 2, TQC], FP8, name=f"ex_{c}_{j}", tag="ex")
                    exs.append(ex)
                nc.scalar.activation(
                    exs[-1][:, j % 2, :],
                    st[:],
                    AT.Exp,
                    bias=bias_sb[:, j : j + 1],
                    scale=SCALE,
                )
                # y tiles of the previous chunk at even j (no exp dependence),
                # spread through this chunk so they don't crowd the mmps PSUM
                # slots at the boundary
                if c > 0 and j in (4, 8, 12):
                    emit_y_tile(c - 1, (4, 8, 12).index(j))
                if j % 2 == 1 and j >= 3:
                    emit_group((j - 3) // 2)
            if c > 0:
                emit_y_tile(c - 1, 3)
            emit_group(NKP - 2)
            emit_group(NKP - 1)
            for i in range(NHT):
                # split across ACT/DVE so neither engine bursts at the boundary
                dst = ot8[i // 2][:, i % 2, c * TQC : (c + 1) * TQC]
                if i % 2 == 0:
                    nc.scalar.copy(dst, otps[i][:])
                else:
                    nc.vector.tensor_copy(dst, otps[i][:])
            sums_sb = sums_pool.tile([1, TQC], F32, name=f"sums_{c}", tag="sums")
            nc.vector.tensor_copy(sums_sb[:], sps[:])
            nc.sync.dma_start(sums_out[c : c + 1, :], sums_sb[:])

        for tl in range(TQC // PART):
            emit_y_tile(NCH - 1, tl)

    nc.compile()
    return nc


_NC_CACHE = None


def get_nc():
    global _NC_CACHE
    if _NC_CACHE is None:
        _NC_CACHE = build_nc()
    return _NC_CACHE


def make_in_maps(rgb, pose, Wq, bq, Wk, bk, Wv, bv, Wp, bp):
    rgb = np.asarray(rgb, np.float32)
    pose = np.asarray(pose, np.float32)
    Wq, bq = np.asarray(Wq, np.float32), np.asarray(bq, np.float32)
    Wk, bk = np.asarray(Wk, np.float32), np.asarray(bk, np.float32)
    Wv = np.asarray(Wv, np.float32)
    Wp = np.asarray(Wp, np.float32)

    xT = np.zeros((B, DRP, T), NP_FP8)
    xT[:, :DR, :] = np.swapaxes(rgb, 1, 2).astype(NP_FP8)
    pT = np.ascontiguousarray(np.swapaxes(pose, 1, 2)).astype(NP_FP8)
    # low-rank score factor M = Wq Wk^T (f32 on host, fp8 on device)
    m8 = np.zeros((DRP, DP), NP_FP8)
    m8[:DR] = (Wq @ Wk.T).astype(NP_FP8)
    wv8 = Wv.astype(NP_FP8)
    wp8 = Wp.astype(NP_FP8)
    # per-key score bias: s2[t] = pose[t] . (Wk bq); exp arg gets
    # SCALE*(u p^T) + SCALE*s2 + EXP_BIAS (query-side bias cancels in
    # the normalization, so it is dropped)
    s2 = pose @ (Wk @ bq)                         # (B, T) f32
    biasd = (s2 * SCALE + EXP_BIAS).reshape(B, NTK, PART)
    biasd = np.ascontiguousarray(np.swapaxes(biasd, 1, 2)).astype(np.float32)
    return [
        dict(
            xT=xT[b], pT=pT[b],
            m8d=m8, wv=wv8, wp=wp8, biasd=biasd[b],
        )
        for b in range(B)
    ]


def kernel(rgb, pose, Wq, bq, Wk, bk, Wv, bv, Wp, bp):
    rgb = np.asarray(rgb, np.float32)
    Wp_f = np.asarray(Wp, np.float32)
    bp_eff = np.asarray(bp, np.float32) + np.asarray(bv, np.float32) @ Wp_f
    in_maps = make_in_maps(rgb, pose, Wq, bq, Wk, bk, Wv, bv, Wp, bp)
    res = bass_utils.run_bass_kernel_spmd(get_nc(), in_maps, core_ids=list(range(B)))
    out = np.empty((B, T, DR), np.float32)
    for b in range(B):
        yun = np.asarray(res.results[b]["yun"], dtype=np.float32)
        sums = res.results[b]["sums_out"].reshape(T)
        out[b] = rgb[b] + bp_eff + yun / sums[:, None]
    return out
